# revision 1
# baseline (speedup 1.0000x reference)
"""Trainium2 Bass kernel for nn_ConvLSTMModel_85418309583256.

Strategy (hardcoded from the reference model structure):
  - replay values are in {0,1}; the categorical/embedding branch reads only
    spatial position (0,0) of each frame, so the [B,T,1927,200] "embedding"
    tensor is emb_table[0] everywhere except at most 5 known channels
    (z-channels 2,6,8,13,15) that flip to emb_table[1] when v0[k]==1.
    conv1 therefore collapses to a K=14 matmul per frame:
      rows 0,1  : W1sum[:,dx] x e0[dx+w]        (W1sum = sum of cnn1_w over emb chans)
      rows 2..11: w1[:,ck,dx] x flip_k*(e1-e0)[dx+w]
      rows 12,13: w1[:,0,dx]  x y[dx+w]         (y = fc_con branch output)
  - data parallel over B*T frames: 8 frames per core on 8 cores.
  - W1sum reduction sharded over channels + AllReduce; seq AllGather; the
    tiny LSTM (hidden 100) runs replicated on every core.
"""

import sys
for _p in ("/opt/trn_rl_repo", "/root/.axon_site/_ro/trn_rl_repo"):
    if _p not in sys.path:
        sys.path.append(_p)

import numpy as np
import concourse.bass as bass
import concourse.bacc as bacc
import concourse.mybir as mybir
from concourse import tile
from concourse.bass_utils import run_bass_kernel_spmd

class _StopEmit(Exception):
    pass


F32 = mybir.dt.float32
AF = mybir.ActivationFunctionType
ALU = mybir.AluOpType

N_CORES = 8
T = 32
# feature indices (of the 8 input features) used by the categorical branch
CAT_FEATS = [1, 3, 5, 6, 7]
# z-space channels (within the 1928-channel CNN input) that flip to e1
CKZ = [2, 6, 8, 13, 15]
# emb-channel count and per-core channel-slab for the W1sum reduction
EMB_CH = 1927
CH_PER_CORE = 241  # 8*241 = 1928 >= 1927, last core padded


def _tiles(total, step=128):
    return [(s, min(step, total - s)) for s in range(0, total, step)]


def _build(B, n_steps=None, stage=99):
    """Build the SPMD bass program for batch size B (F = B*T//8 frames/core)."""
    if n_steps is None:
        n_steps = T
    BT = B * T
    assert BT % N_CORES == 0
    F = BT // N_CORES
    GW = B  # gate width in the LSTM tiles

    nc = bacc.Bacc("TRN2", target_bir_lowering=False, debug=False,
                   num_devices=N_CORES)

    def din(name, shape):
        return nc.dram_tensor(name, list(shape), F32, kind="ExternalInput")

    # ---- per-core external inputs -------------------------------------
    scal = din("scal", (F, 128, 128))
    v0dup = din("v0dup", (10, F))
    w1slice = din("w1slice", (964, 2 * CH_PER_CORE))
    w1colsT = din("w1colsT", (12, 964))
    emb01 = din("emb01", (2, 200))
    emb10 = din("emb10", (2, 200))
    ccw = din("ccw", (128, 16))
    ccb = din("ccb", (128, 1))
    fcconT = din("fcconT", (961, 200))
    fcconb = din("fcconb", (100, 2))
    cnn1b = din("cnn1b", (128, 8))
    w2T = din("w2T", (2, 964, 482))
    cnn2b = din("cnn2b", (128, 4))
    w3T = din("w3T", (2, 482, 241))
    cnn3b = din("cnn3b", (128, 2))
    w4T = din("w4T", (2, 241, 100))
    cnn4b = din("cnn4b", (100, 1))
    fccnnT = din("fccnnT", (2300, 100))   # rows 100*w + c
    fccnnb = din("fccnnb", (100, 1))
    wihT = din("wihT", (100, 400))        # gate order (i,f,o,g)
    whhT = din("whhT", (100, 400))
    lstmb = din("lstmb", (100, 4))        # b_ih+b_hh is NOT summed on host; see below
    lstmb2 = din("lstmb2", (100, 4))
    fcoT = din("fcoT", (100, 2))
    fcob = din("fcob", (2, 1))
    ident = din("ident", (128, 128))

    out = nc.dram_tensor("out", [B, 2], F32, kind="ExternalOutput")

    M964 = _tiles(964)
    M482 = _tiles(482)
    M241 = _tiles(241)
    K961 = _tiles(961, 124)  # 7x124 + 93, aligned to whole p-rows (31 each 4)

    def _emit():
        with tile.TileContext(nc) as tc:
            with (
                tc.tile_pool(name="cst", bufs=1) as cp,
                tc.tile_pool(name="wrk", bufs=1) as wp,
                tc.tile_pool(name="cps", bufs=3, space="PSUM") as pca,
                tc.tile_pool(name="sps", bufs=2, space="PSUM") as pcb,
                tc.tile_pool(name="dram", bufs=1, space="DRAM") as dp,
            ):
                def load(name, dram, shape=None, sl=None, eng=None):
                    t = cp.tile(list(shape if shape else dram.shape), F32, tag=name)
                    (eng or nc.sync).dma_start(t[:], dram.ap() if sl is None else sl)
                    return t

                # w1slice first on the SP queue: it gates the AllReduce, the
                # longest-latency chain in the kernel.
                w1s_sb = [cp.tile([msz, 2 * CH_PER_CORE], F32, tag=f"w1s_{mt}", name=f"w1s_{mt}")
                          for mt, (ms, msz) in enumerate(M964)]
                for mt, (ms, msz) in enumerate(M964):
                    nc.sync.dma_start(w1s_sb[mt][:], w1slice.ap()[ms:ms + msz, :])
                # small consts on the ACT queue (fast, needed early)
                ident_sb = load("ident", ident, eng=nc.scalar)
                emb_sb = load("emb", emb01, eng=nc.scalar)
                emb1_sb = load("emb1", emb10, eng=nc.scalar)
                ccw_sb = load("ccw", ccw, eng=nc.scalar)
                ccb_sb = load("ccb", ccb, eng=nc.scalar)
                v0_sb = load("v0", v0dup, eng=nc.scalar)
                c1b_sb = load("c1b", cnn1b, eng=nc.scalar)
                c2b_sb = load("c2b", cnn2b, eng=nc.scalar)
                c3b_sb = load("c3b", cnn3b, eng=nc.scalar)
                c4b_sb = load("c4b", cnn4b, eng=nc.scalar)
                fcb_sb = load("fcb", fcconb, eng=nc.scalar)
                fnb_sb = load("fnb", fccnnb, eng=nc.scalar)
                # frame rows for conv_con on the ACT queue too
                row_tiles = []
                sv = scal.ap().rearrange("f (r c2) c -> r c2 f c", c2=2)
                for r in range(4):
                    rt = cp.tile([63, F, 128], F32, tag=f"rows{r}", name=f"rows{r}")
                    nc.scalar.dma_start(rt[:], sv[r // 2:r // 2 + 63, r % 2])
                    row_tiles.append(rt)
                fcc_sb = [cp.tile([ksz, 200], F32, tag=f"fcc_{t_}", name=f"fcc_{t_}")
                          for t_, (ks, ksz) in enumerate(K961)]
                for t_, (ks, ksz) in enumerate(K961):
                    nc.scalar.dma_start(fcc_sb[t_][:], fcconT.ap()[ks:ks + ksz, :])
                # heavy conv weights stream on the ACT queue (needed later)
                w2_sb = [[cp.tile([ksz, 482], F32, tag=f"w2_{dx}_{kt}", name=f"w2_{dx}_{kt}")
                          for kt, (ks, ksz) in enumerate(M964)] for dx in range(2)]
                for dx in range(2):
                    for kt, (ks, ksz) in enumerate(M964):
                        nc.scalar.dma_start(w2_sb[dx][kt][:],
                                            w2T.ap()[dx, ks:ks + ksz, :])
                w3_sb = [[cp.tile([ksz, 241], F32, tag=f"w3_{dx}_{kt}", name=f"w3_{dx}_{kt}")
                          for kt, (ks, ksz) in enumerate(M482)] for dx in range(2)]
                for dx in range(2):
                    for kt, (ks, ksz) in enumerate(M482):
                        nc.scalar.dma_start(w3_sb[dx][kt][:],
                                            w3T.ap()[dx, ks:ks + ksz, :])
                w4_sb = [[cp.tile([ksz, 100], F32, tag=f"w4_{dx}_{kt}", name=f"w4_{dx}_{kt}")
                          for kt, (ks, ksz) in enumerate(M241)] for dx in range(2)]
                for dx in range(2):
                    for kt, (ks, ksz) in enumerate(M241):
                        nc.scalar.dma_start(w4_sb[dx][kt][:],
                                            w4T.ap()[dx, ks:ks + ksz, :])
                fcnn_sb = cp.tile([100, 23, 100], F32, tag="fcnn", name="fcnn")
                nc.scalar.dma_start(
                    fcnn_sb[:],
                    fccnnT.ap().rearrange("(w c) o -> c w o", c=100))
                wih_sb = load("wih", wihT, eng=nc.scalar)
                whh_sb = load("whh", whhT, eng=nc.scalar)
                bih_sb = load("bih", lstmb, eng=nc.scalar)
                bhh_sb = load("bhh", lstmb2, eng=nc.scalar)
                fco_sb = load("fco", fcoT, eng=nc.scalar)
                fob_sb = load("fob", fcob, eng=nc.scalar)

                if stage < 2:
                    return
                # ---- W1sum: shard-reduce cnn1_w over emb channels + AllReduce
                # lhsT row order: 0..9 gated (k,dx); 10,11 W1sum dx; 12,13 sca dx
                lhsT = cp.tile([14, 964], F32, tag="lhsT", name="lhsT")
                nc.sync.dma_start(lhsT[0:10, :], w1colsT.ap()[0:10, :])
                nc.sync.dma_start(lhsT[12:14, :], w1colsT.ap()[10:12, :])
                w1part = [wp.tile([msz, 2], F32, tag=f"w1p_{mt}", name=f"w1p_{mt}")
                          for mt, (ms, msz) in enumerate(M964)]
                w1p_b = dp.tile([964, 2], F32, tag="w1p_b", name="w1p_b")
                w1r_b = dp.tile([964, 2], F32, tag="w1r_b", name="w1r_b")
                for mt, (ms, msz) in enumerate(M964):
                    v = w1s_sb[mt][:].rearrange("m (c two) -> m c two", two=2)
                    for dx in range(2):
                        nc.vector.tensor_reduce(
                            w1part[mt][:, dx:dx + 1], v[:, :, dx],
                            axis=mybir.AxisListType.X, op=ALU.add)
                    nc.sync.dma_start(w1p_b[ms:ms + msz, :], w1part[mt][:])
                nc.gpsimd.collective_compute(
                    "AllReduce", ALU.add,
                    replica_groups=[list(range(N_CORES))],
                    ins=[w1p_b.opt()], outs=[w1r_b.opt()])
                for mt, (ms, msz) in enumerate(M964):
                    w1sum_t = wp.tile([msz, 2], F32, tag=f"w1sum_{mt}", name=f"w1sum_{mt}")
                    nc.sync.dma_start(w1sum_t[:], w1r_b[ms:ms + msz, :])
                    tp = pcb.tile([2, msz], F32, tag="sps", name="sps")
                    nc.tensor.transpose(tp[:], w1sum_t[:], ident_sb[0:msz, 0:msz])
                    w1t_sb = wp.tile([2, msz], F32, tag="w1t", name="w1t", bufs=2)
                    nc.vector.tensor_copy(w1t_sb[:], tp[:])
                    nc.sync.dma_start(lhsT[10:12, ms:ms + msz], w1t_sb[:])

                if stage < 3:
                    return
                # ---- scalar branch: conv_con 4x4 stride2 + relu + pool ------
                acc = wp.tile([63, F, 63], F32, tag="acc", name="acc")
                for r in range(4):
                    rar = row_tiles[r][:, :, :].rearrange(
                        "p f (w two) -> p f w two", two=2)
                    for cc in range(4):
                        u, vv = cc // 2, cc % 2
                        view = rar[:, :, u:u + 63, vv]
                        widx = 4 * r + cc
                        if r == 0 and cc == 0:
                            nc.vector.tensor_scalar(
                                out=acc[:], in0=view,
                                scalar1=ccw_sb[0:63, widx:widx + 1],
                                scalar2=None, op0=ALU.mult)
                        else:
                            nc.vector.scalar_tensor_tensor(
                                out=acc[:], in0=view,
                                scalar=ccw_sb[0:63, widx:widx + 1],
                                in1=acc[:], op0=ALU.mult, op1=ALU.add)
                # cp_t free layout is (q, f) so the flatten-shuffle DMA below
                # reads each partition row contiguously
                cp_t = wp.tile([63, 31, F], F32, tag="cpool", name="cpool")
                av = acc[:, :, 0:62].rearrange("p f (w two) -> p f w two", two=2)
                nc.vector.tensor_tensor(out=cp_t[:].rearrange("p q f -> p f q"),
                                        in0=av[:, :, :, 0],
                                        in1=av[:, :, :, 1], op=ALU.max)
                nc.vector.tensor_scalar(out=cp_t[:], in0=cp_t[:],
                                        scalar1=ccb_sb[0:63, 0:1], op0=ALU.add,
                                        scalar2=0.0, op1=ALU.max)
                # row pool + flatten shuffle into fc_con rhs K-tiles
                rhsfc = [wp.tile([ksz, F], F32, tag=f"rhsfc_{t_}", name=f"rhsfc_{t_}")
                         for t_, (ks, ksz) in enumerate(K961)]
                for t_, (ks, ksz) in enumerate(K961):
                    np_rows = ksz // 31  # 4 or 3 p-rows in this tile
                    e2 = wp.tile([ksz, F], F32, tag=f"e2_{t_}", name=f"e2_{t_}")
                    o2 = wp.tile([ksz, F], F32, tag=f"o2_{t_}", name=f"o2_{t_}")
                    for pl in range(np_rows):
                        p = 4 * t_ + pl
                        for half, dst in ((0, e2), (1, o2)):
                            src = cp_t[2 * p + half: 2 * p + half + 1, :, :]
                            # [1 part, (q,f) contiguous] -> [31 part, F]
                            eng = nc.sync if half == 0 else nc.scalar
                            eng.dma_start(
                                dst[31 * pl:31 * pl + 31, :],
                                src.rearrange("one q f -> one (q f)"))
                    nc.vector.tensor_tensor(out=rhsfc[t_][:], in0=e2[:],
                                            in1=o2[:], op=ALU.max)
                # fc_con GEMM -> yT [200, F] as two [100, F] tiles
                yT = []
                for m2 in range(2):
                    yps = pcb.tile([100, F], F32, tag="yps", name="yps")
                    for t_, (ks, ksz) in enumerate(K961):
                        nc.tensor.matmul(
                            yps[:], fcc_sb[t_][:, 100 * m2:100 * m2 + 100],
                            rhsfc[t_][:],
                            start=(t_ == 0), stop=(t_ == len(K961) - 1))
                    yt = wp.tile([100, F], F32, tag=f"yT_{m2}", name=f"yT_{m2}")
                    nc.scalar.activation(yt[:], yps[:], AF.Identity,
                                         bias=fcb_sb[:, m2:m2 + 1])
                    yT.append(yt)
                y_sb = wp.tile([F, 200], F32, tag="y", name="y")
                for m2 in range(2):
                    tp = pcb.tile([F, 100], F32, tag="sps", name="sps")
                    nc.tensor.transpose(tp[:], yT[m2][:], ident_sb[0:100, 0:100])
                    nc.vector.tensor_copy(y_sb[:, 100 * m2:100 * m2 + 100], tp[:])

                if stage < 5:
                    return
                # ---- rhs assembly for the collapsed conv1 ------------------
                # rhs row order matches lhsT: 0..9 gated; 10,11 e0; 12,13 y
                rhs = wp.tile([14, F, 199], F32, tag="rhs", name="rhs")
                for dx in range(2):
                    for f in range(F):
                        eng = nc.sync if f % 2 == 0 else nc.scalar
                        eng.dma_start(rhs[10 + dx:11 + dx, f, :],
                                      emb01.ap()[0:1, dx:dx + 199])
                d_t = wp.tile([1, 200], F32, tag="d", name="d")
                nc.vector.tensor_tensor(out=d_t[:], in0=emb1_sb[0:1, :],
                                        in1=emb_sb[0:1, :], op=ALU.subtract)
                dtile = wp.tile([10, 199], F32, tag="dtile", name="dtile")
                for k in range(5):
                    for dx in range(2):
                        nc.sync.dma_start(dtile[2 * k + dx:2 * k + dx + 1, :],
                                          d_t[0:1, dx:dx + 199])
                flip = wp.tile([10, F], F32, tag="flip", name="flip")
                nc.vector.tensor_scalar(out=flip[:], in0=v0_sb[:], scalar1=1.0,
                                        scalar2=None, op0=ALU.is_equal)
                for f in range(F):
                    nc.vector.tensor_scalar(out=rhs[0:10, f, :], in0=dtile[:],
                                            scalar1=flip[:, f:f + 1],
                                            scalar2=None, op0=ALU.mult)
                for dx in range(2):
                    nc.sync.dma_start(rhs[12 + dx:13 + dx, :, :],
                                      y_sb[:, dx:dx + 199])

                if stage < 6:
                    return
                # ---- conv1 (collapsed) + pool -> pooled1 -------------------
                pooled1 = [cp.tile([msz, F, 99], F32, tag=f"p1_{mt}", name=f"p1_{mt}")
                           for mt, (ms, msz) in enumerate(M964)]
                for mt, (ms, msz) in enumerate(M964):
                    for fg in range(0, F, 2):
                        nf = min(2, F - fg)
                        ps = pca.tile([msz, nf, 199], F32, tag="cps", name="cps")
                        nc.tensor.matmul(ps[:], lhsT[:, ms:ms + msz],
                                         rhs[:, fg:fg + nf, :],
                                         start=True, stop=True)
                        # pool+bias+relu on DVE (only one psum operand per op)
                        ev = ps[:, :, 0:198].rearrange(
                            "m f (w two) -> m f w two", two=2)
                        dst = pooled1[mt][:, fg:fg + nf, :]
                        todd1 = wp.tile([msz, nf, 99], F32, tag="c1tmp",
                                        name="c1tmp", bufs=2)
                        nc.vector.tensor_copy(todd1[:], ev[:, :, :, 1])
                        nc.vector.tensor_tensor(out=dst, in0=ev[:, :, :, 0],
                                                in1=todd1[:], op=ALU.max)
                        nc.vector.tensor_scalar(out=dst, in0=dst,
                                                scalar1=c1b_sb[0:msz, mt:mt + 1],
                                                op0=ALU.add, scalar2=0.0,
                                                op1=ALU.max)

                if stage < 7:
                    return
                # ---- conv2 + pool -> pooled2 -------------------------------
                pooled2 = [cp.tile([msz, F, 49], F32, tag=f"p2_{mt}", name=f"p2_{mt}")
                           for mt, (ms, msz) in enumerate(M482)]
                for mt, (ms, msz) in enumerate(M482):
                    for fg in range(0, F, 4):
                        nf = min(4, F - fg)
                        ps = pca.tile([msz, nf, 98], F32, tag="cps", name="cps")
                        nmm = len(M964) * 2
                        i = 0
                        for kt, (ks, ksz) in enumerate(M964):
                            for dx in range(2):
                                nc.tensor.matmul(
                                    ps[:], w2_sb[dx][kt][:, ms:ms + msz],
                                    pooled1[kt][:, fg:fg + nf, dx:dx + 98],
                                    start=(i == 0), stop=(i == nmm - 1))
                                i += 1
                        ev = ps[:, :, 0:98].rearrange(
                            "m f (w two) -> m f w two", two=2)
                        dst = pooled2[mt][:, fg:fg + nf, :]
                        todd = wp.tile([msz, nf, 49], F32, tag="c2tmp",
                                       name="c2tmp", bufs=2)
                        nc.vector.tensor_copy(todd[:], ev[:, :, :, 1])
                        nc.vector.tensor_tensor(out=dst, in0=ev[:, :, :, 0],
                                                in1=todd[:], op=ALU.max)
                        nc.vector.tensor_scalar(out=dst, in0=dst,
                                                scalar1=c2b_sb[0:msz, mt:mt + 1],
                                                op0=ALU.add, scalar2=0.0,
                                                op1=ALU.max)

                if stage < 8:
                    return
                # ---- conv3 + pool -> pooled3 -------------------------------
                pooled3 = [cp.tile([msz, F, 24], F32, tag=f"p3_{mt}", name=f"p3_{mt}")
                           for mt, (ms, msz) in enumerate(M241)]
                for mt, (ms, msz) in enumerate(M241):
                    ps = pca.tile([msz, F, 48], F32, tag="cps", name="cps")
                    nmm = len(M482) * 2
                    i = 0
                    for kt, (ks, ksz) in enumerate(M482):
                        for dx in range(2):
                            nc.tensor.matmul(
                                ps[:], w3_sb[dx][kt][:, ms:ms + msz],
                                pooled2[kt][:, :, dx:dx + 48],
                                start=(i == 0), stop=(i == nmm - 1))
                            i += 1
                    ev = ps[:, :, 0:48].rearrange("m f (w two) -> m f w two", two=2)
                    dst = pooled3[mt][:]
                    todd3 = wp.tile([msz, F, 24], F32, tag="c3tmp",
                                    name="c3tmp", bufs=2)
                    nc.vector.tensor_copy(todd3[:], ev[:, :, :, 1])
                    nc.vector.tensor_tensor(out=dst, in0=ev[:, :, :, 0],
                                            in1=todd3[:], op=ALU.max)
                    nc.vector.tensor_scalar(out=dst, in0=dst,
                                            scalar1=c3b_sb[0:msz, mt:mt + 1],
                                            op0=ALU.add, scalar2=0.0, op1=ALU.max)

                if stage < 8:
                    return
                # ---- conv4 + relu -> conv4o [100, F, 23] -------------------
                conv4o = wp.tile([100, F, 23], F32, tag="c4o", name="c4o")
                ps4 = pca.tile([100, F, 23], F32, tag="cps", name="cps")
                i = 0
                for kt, (ks, ksz) in enumerate(M241):
                    for dx in range(2):
                        nc.tensor.matmul(ps4[:], w4_sb[dx][kt][:],
                                         pooled3[kt][:, :, dx:dx + 23],
                                         start=(i == 0), stop=(i == 3))
                        i += 1
                nc.scalar.activation(conv4o[:], ps4[:], AF.Relu,
                                     bias=c4b_sb[:, 0:1])

                if stage < 8:
                    return
                # ---- fc_cnn -> seqT_j [100, F]; AllGather -> seqT [100, BT]
                agin = dp.tile([100, F], F32, tag="agin", name="agin")
                agout = dp.tile([N_CORES * 100, F], F32, tag="agout",
                                addr_space="Shared")
                ps5 = pcb.tile([100, F], F32, tag="yps", name="yps")
                for w in range(23):
                    nc.tensor.matmul(ps5[:], fcnn_sb[:, w, :], conv4o[:, :, w],
                                     start=(w == 0), stop=(w == 22))
                seqj = wp.tile([100, F], F32, tag="seqj", name="seqj")
                nc.scalar.activation(seqj[:], ps5[:], AF.Identity,
                                     bias=fnb_sb[:, 0:1])
                nc.sync.dma_start(agin[:], seqj[:])
                nc.gpsimd.collective_compute(
                    "AllGather", ALU.bypass,
                    replica_groups=[list(range(N_CORES))],
                    ins=[agin.opt()], outs=[agout.opt()])
                seqT = wp.tile([100, BT], F32, tag="seqT", name="seqT")
                for j in range(N_CORES):
                    nc.sync.dma_start(seqT[:, F * j:F * j + F],
                                      agout[100 * j:100 * j + 100, :])

                if stage < 9:
                    return
                # ---- LSTM (replicated) -------------------------------------
                badd = wp.tile([100, 4], F32, tag="badd", name="badd")
                nc.vector.tensor_tensor(out=badd[:], in0=bih_sb[:], in1=bhh_sb[:],
                                        op=ALU.add)
                xpall = wp.tile([100, T, 4 * GW], F32, tag="xpall", name="xpall")
                for g in range(4):
                    xps = pcb.tile([100, BT], F32, tag="sps", name="sps")
                    nc.tensor.matmul(xps[:], wih_sb[:, 100 * g:100 * g + 100],
                                     seqT[:], start=True, stop=True)
                    xp = wp.tile([100, B, T], F32, tag="xp", name="xp")
                    nc.scalar.activation(xp[:], xps[:].rearrange("p (b t) -> p b t", b=B),
                                         AF.Identity, bias=badd[:, g:g + 1])
                    nc.vector.tensor_copy(xpall[:, :, GW * g:GW * g + GW],
                                          xp[:].rearrange("p b t -> p t b"))
                h_t = wp.tile([100, GW], F32, tag="h", name="h")
                c_t = wp.tile([100, GW], F32, tag="c", name="c")
                nc.vector.memset(h_t[:], 0.0)
                nc.vector.memset(c_t[:], 0.0)
                for t_ in range(n_steps):
                    gps = pcb.tile([100, 4 * GW], F32, tag="sps", name="sps")
                    for g in range(4):
                        nc.tensor.matmul(gps[:, GW * g:GW * g + GW],
                                         whh_sb[:, 100 * g:100 * g + 100],
                                         h_t[:], start=True, stop=True)
                    gsb = wp.tile([100, 4 * GW], F32, tag="gsb", name="gsb")
                    nc.vector.tensor_tensor(out=gsb[:], in0=gps[:],
                                            in1=xpall[:, t_, :], op=ALU.add)
                    acts = wp.tile([100, 4 * GW], F32, tag="acts", name="acts")
                    nc.scalar.activation(acts[:, 0:3 * GW], gsb[:, 0:3 * GW],
                                         AF.Sigmoid)
                    nc.scalar.activation(acts[:, 3 * GW:4 * GW],
                                         gsb[:, 3 * GW:4 * GW], AF.Tanh)
                    t1 = wp.tile([100, GW], F32, tag="t1", name="t1")
                    t2 = wp.tile([100, GW], F32, tag="t2", name="t2")
                    nc.vector.tensor_tensor(out=t1[:], in0=acts[:, 0:GW],
                                            in1=acts[:, 3 * GW:4 * GW],
                                            op=ALU.mult)
                    nc.vector.tensor_tensor(out=t2[:], in0=acts[:, GW:2 * GW],
                                            in1=c_t[:], op=ALU.mult)
                    nc.vector.tensor_tensor(out=c_t[:], in0=t1[:], in1=t2[:],
                                            op=ALU.add)
                    th = wp.tile([100, GW], F32, tag="th", name="th")
                    nc.scalar.activation(th[:], c_t[:], AF.Tanh)
                    nc.vector.tensor_tensor(out=h_t[:], in0=acts[:, 2 * GW:3 * GW],
                                            in1=th[:], op=ALU.mult)
                ops = pcb.tile([B, 2], F32, tag="sps", name="sps")
                nc.tensor.matmul(ops[:], h_t[:], fco_sb[:], start=True, stop=True)
                out_sb = wp.tile([B, 2], F32, tag="out", name="out")
                nc.scalar.activation(out_sb[:], ops[:], AF.Identity,
                                     bias=fob_sb[0:B, 0:1])
                nc.sync.dma_start(out.ap(), out_sb[:])

    _emit()
    nc.compile()
    return nc


_CACHE = {}


def _get_nc(B):
    if B not in _CACHE:
        _CACHE[B] = _build(B)
    return _CACHE[B]


def _prep_inputs(B, **inp):
    """Host-side sharding/layout prep. Pure slicing/transpose/reshape."""
    f32 = np.float32
    T_ = T
    replay = np.asarray(inp["replay_tensor"], f32)[:B]
    BT = B * T_
    F = BT // N_CORES
    frames = replay.reshape(BT, 8, 128, 128)
    scal_all = np.ascontiguousarray(frames[:, 0])              # [BT,128,128]
    v0_all = np.ascontiguousarray(frames[:, CAT_FEATS, 0, 0])  # [BT,5]

    emb = np.asarray(inp["emb_table"], f32)
    w1 = np.asarray(inp["cnn1_w"], f32)[:, :, 0, :]            # [964,1928,2]
    w1colsT = np.stack(
        [w1[:, c, d] for c in CKZ for d in (0, 1)]
        + [w1[:, 0, d] for d in (0, 1)]).astype(f32)           # [12,964]

    def padbias(b, mts, cols):
        b = np.asarray(b, f32)
        o = np.zeros((128, cols), f32)
        for i, (s, sz) in enumerate(mts):
            o[:sz, i] = b[s:s + sz]
        return o

    ccw_bc = np.broadcast_to(
        np.asarray(inp["conv_con_w"], f32).reshape(16)[None, :],
        (128, 16)).copy()
    ccb_bc = np.full((128, 1), np.asarray(inp["conv_con_b"], f32).reshape(1)[0],
                     f32)
    fcconT = np.ascontiguousarray(np.asarray(inp["fc_con_w"], f32).T)  # [961,200]
    fcconb = np.ascontiguousarray(
        np.asarray(inp["fc_con_b"], f32).reshape(2, 100).T)            # [100,2]
    w2T = np.ascontiguousarray(
        np.asarray(inp["cnn2_w"], f32)[:, :, 0, :].transpose(2, 1, 0))  # [2,964,482]
    w3T = np.ascontiguousarray(
        np.asarray(inp["cnn3_w"], f32)[:, :, 0, :].transpose(2, 1, 0))
    w4T = np.ascontiguousarray(
        np.asarray(inp["cnn4_w"], f32)[:, :, 0, :].transpose(2, 1, 0))
    fccnnT = np.ascontiguousarray(
        np.asarray(inp["fc_cnn_w"], f32).reshape(100, 100, 23)
        .transpose(2, 1, 0).reshape(2300, 100))                # rows 100w+c
    perm = np.r_[0:100, 100:200, 300:400, 200:300]             # (i,f,o,g)
    wihT = np.ascontiguousarray(np.asarray(inp["lstm_w_ih"], f32)[perm].T)
    whhT = np.ascontiguousarray(np.asarray(inp["lstm_w_hh"], f32)[perm].T)
    bih = np.ascontiguousarray(
        np.asarray(inp["lstm_b_ih"], f32)[perm].reshape(4, 100).T)
    bhh = np.ascontiguousarray(
        np.asarray(inp["lstm_b_hh"], f32)[perm].reshape(4, 100).T)
    fcoT = np.ascontiguousarray(np.asarray(inp["fc_out_w"], f32).T)  # [100,2]
    fcob = np.asarray(inp["fc_out_b"], f32).reshape(2, 1)

    common = dict(
        w1colsT=w1colsT,
        emb01=np.ascontiguousarray(emb[0:2]),
        emb10=np.ascontiguousarray(emb[[1, 0]]),
        ccw=ccw_bc, ccb=ccb_bc,
        fcconT=fcconT, fcconb=fcconb,
        cnn1b=padbias(inp["cnn1_b"], _tiles(964), 8),
        w2T=w2T, cnn2b=padbias(inp["cnn2_b"], _tiles(482), 4),
        w3T=w3T, cnn3b=padbias(inp["cnn3_b"], _tiles(241), 2),
        w4T=w4T, cnn4b=np.asarray(inp["cnn4_b"], f32).reshape(100, 1),
        fccnnT=fccnnT,
        fccnnb=np.asarray(inp["fc_cnn_b"], f32).reshape(100, 1),
        wihT=wihT, whhT=whhT, lstmb=bih, lstmb2=bhh,
        fcoT=fcoT, fcob=fcob,
        ident=np.eye(128, dtype=f32),
    )

    in_maps = []
    for j in range(N_CORES):
        lo = 1 + CH_PER_CORE * j
        hi = min(lo + CH_PER_CORE, 1928)
        sl = w1[:, lo:hi, :]
        if hi - lo < CH_PER_CORE:
            sl = np.concatenate(
                [sl, np.zeros((964, CH_PER_CORE - (hi - lo), 2), f32)], axis=1)
        m = dict(common)
        m["scal"] = np.ascontiguousarray(scal_all[F * j:F * j + F])
        m["v0dup"] = np.ascontiguousarray(
            np.repeat(v0_all[F * j:F * j + F].T, 2, axis=0))   # [10,F]
        m["w1slice"] = np.ascontiguousarray(sl.reshape(964, 2 * CH_PER_CORE))
        in_maps.append(m)
    return in_maps


def kernel(**inputs):
    B = int(np.asarray(inputs.get("batch_size", 2)))
    if B <= 0:
        return np.zeros((0, 2), np.float32)
    nc = _get_nc(B)
    in_maps = _prep_inputs(B, **inputs)
    res = run_bass_kernel_spmd(nc, in_maps, list(range(N_CORES)))
    return np.asarray(res.results[0]["out"], np.float32)



# revision 10
# speedup vs baseline: 1.2689x; 1.2689x over previous
"""Trainium2 Bass kernel for nn_ConvLSTMModel_85418309583256.

Strategy (hardcoded from the reference model structure):
  - replay values are in {0,1}; the categorical/embedding branch reads only
    spatial position (0,0) of each frame, so the [B,T,1927,200] "embedding"
    tensor is emb_table[0] everywhere except at most 5 known channels
    (z-channels 2,6,8,13,15) that flip to emb_table[1] when v0[k]==1.
    conv1 therefore collapses to a K=14 matmul per frame:
      rows 0,1  : W1sum[:,dx] x e0[dx+w]        (W1sum = sum of cnn1_w over emb chans)
      rows 2..11: w1[:,ck,dx] x flip_k*(e1-e0)[dx+w]
      rows 12,13: w1[:,0,dx]  x y[dx+w]         (y = fc_con branch output)
  - data parallel over B*T frames: 8 frames per core on 8 cores.
  - W1sum reduction sharded over channels + AllReduce; seq AllGather; the
    tiny LSTM (hidden 100) runs replicated on every core.
"""

import sys
for _p in ("/opt/trn_rl_repo", "/root/.axon_site/_ro/trn_rl_repo"):
    if _p not in sys.path:
        sys.path.append(_p)

import numpy as np
import concourse.bass as bass
import concourse.bacc as bacc
import concourse.mybir as mybir
from concourse import tile
from concourse.bass_utils import run_bass_kernel_spmd

class _StopEmit(Exception):
    pass


F32 = mybir.dt.float32
F32R = mybir.dt.float32r
AF = mybir.ActivationFunctionType
ALU = mybir.AluOpType


def _r(ap):
    return ap.bitcast(F32R)

N_CORES = 8
T = 32
# feature indices (of the 8 input features) used by the categorical branch
CAT_FEATS = [1, 3, 5, 6, 7]
# z-space channels (within the 1928-channel CNN input) that flip to e1
CKZ = [2, 6, 8, 13, 15]
# emb-channel count and per-core channel-slab for the W1sum reduction
EMB_CH = 1927
CH_PER_CORE = 241  # 8*241 = 1928 >= 1927, last core padded


def _tiles(total, step=128):
    return [(s, min(step, total - s)) for s in range(0, total, step)]


def _build(B, n_steps=None, stage=99):
    """Build the SPMD bass program for batch size B (F = B*T//8 frames/core)."""
    if n_steps is None:
        n_steps = T
    BT = B * T
    assert BT % N_CORES == 0
    F = BT // N_CORES
    GW = B  # gate width in the LSTM tiles

    nc = bacc.Bacc("TRN2", target_bir_lowering=False, debug=False,
                   num_devices=N_CORES)

    def din(name, shape, dt=F32):
        return nc.dram_tensor(name, list(shape), dt, kind="ExternalInput")

    # ---- per-core external inputs -------------------------------------
    scal = din("scal", (F, 128, 128))
    v0dup = din("v0dup", (10, F))
    w1slice = din("w1slice", (964, 2 * CH_PER_CORE))
    w1colsT = din("w1colsT", (12, 964), F32R)
    emb01 = din("emb01", (2, 200))
    emb10 = din("emb10", (2, 200))
    ccw = din("ccw", (128, 16))
    ccb = din("ccb", (128, 1))
    fcconT = din("fcconT", (961, 200))
    fcconb = din("fcconb", (100, 2))
    cnn1b = din("cnn1b", (128, 8))
    w2T = din("w2T", (2, 964, 482), F32R)
    cnn2b = din("cnn2b", (128, 4))
    w3T = din("w3T", (2, 482, 241), F32R)
    cnn3b = din("cnn3b", (128, 2))
    w4T = din("w4T", (2, 241, 100))
    cnn4b = din("cnn4b", (100, 1))
    fccnnT = din("fccnnT", (2300, 100))   # rows 100*w + c
    fccnnb = din("fccnnb", (100, 1))
    wihT = din("wihT", (100, 400))        # gate order (i,f,o,g)
    whhT = din("whhT", (100, 400))
    lstmb = din("lstmb", (100, 4))        # b_ih+b_hh is NOT summed on host; see below
    lstmb2 = din("lstmb2", (100, 4))
    fcoT = din("fcoT", (100, 2))
    fcob = din("fcob", (2, 1))
    ident = din("ident", (128, 128))

    out = nc.dram_tensor("out", [B, 2], F32, kind="ExternalOutput")

    M964 = _tiles(964)
    M482 = _tiles(482)
    M241 = _tiles(241)
    K961 = _tiles(961, 124)  # 7x124 + 93, aligned to whole p-rows (31 each 4)

    def _emit():
        with tile.TileContext(nc) as tc:
            with (
                tc.tile_pool(name="cst", bufs=1) as cp,
                tc.tile_pool(name="wrk", bufs=1) as wp,
                tc.tile_pool(name="cps", bufs=3, space="PSUM") as pca,
                tc.tile_pool(name="sps", bufs=2, space="PSUM") as pcb,
                tc.tile_pool(name="dram", bufs=1, space="DRAM") as dp,
            ):
                def load(name, dram, shape=None, sl=None, eng=None):
                    t = cp.tile(list(shape if shape else dram.shape), F32, tag=name)
                    (eng or nc.sync).dma_start(t[:], dram.ap() if sl is None else sl)
                    return t

                # w1slice first on the SP queue: it gates the AllReduce, the
                # longest-latency chain in the kernel.
                w1s_sb = [cp.tile([msz, 2 * CH_PER_CORE], F32, tag=f"w1s_{mt}", name=f"w1s_{mt}")
                          for mt, (ms, msz) in enumerate(M964)]
                for mt, (ms, msz) in enumerate(M964):
                    nc.sync.dma_start(w1s_sb[mt][:], w1slice.ap()[ms:ms + msz, :])
                # small consts on the ACT queue (fast, needed early)
                ident_sb = load("ident", ident, eng=nc.scalar)
                emb_sb = load("emb", emb01, eng=nc.scalar)
                emb1_sb = load("emb1", emb10, eng=nc.scalar)
                ccw_sb = load("ccw", ccw, eng=nc.scalar)
                ccb_sb = load("ccb", ccb, eng=nc.scalar)
                v0_sb = load("v0", v0dup, eng=nc.scalar)
                c1b_sb = load("c1b", cnn1b, eng=nc.scalar)
                c2b_sb = load("c2b", cnn2b, eng=nc.scalar)
                c3b_sb = load("c3b", cnn3b, eng=nc.scalar)
                c4b_sb = load("c4b", cnn4b, eng=nc.scalar)
                fcb_sb = load("fcb", fcconb, eng=nc.scalar)
                fnb_sb = load("fnb", fccnnb, eng=nc.scalar)
                # frame rows for conv_con on the ACT queue too
                row_tiles = []
                sv = scal.ap().rearrange("f (r c2) c -> r c2 f c", c2=2)
                for r in range(4):
                    rt = cp.tile([63, F, 128], F32, tag=f"rows{r}", name=f"rows{r}")
                    nc.scalar.dma_start(rt[:], sv[r // 2:r // 2 + 63, r % 2])
                    row_tiles.append(rt)
                fcc_sb = [cp.tile([ksz, 200], F32, tag=f"fcc_{t_}", name=f"fcc_{t_}")
                          for t_, (ks, ksz) in enumerate(K961)]
                for t_, (ks, ksz) in enumerate(K961):
                    nc.scalar.dma_start(fcc_sb[t_][:], fcconT.ap()[ks:ks + ksz, :])
                # heavy conv weights stream on the ACT queue (needed later)
                w2_sb = [[cp.tile([ksz, 482], F32R, tag=f"w2_{dx}_{kt}", name=f"w2_{dx}_{kt}")
                          for kt, (ks, ksz) in enumerate(M964)] for dx in range(2)]
                for dx in range(2):
                    for kt, (ks, ksz) in enumerate(M964):
                        nc.scalar.dma_start(w2_sb[dx][kt][:],
                                            w2T.ap()[dx, ks:ks + ksz, :])
                w3_sb = [[cp.tile([ksz, 241], F32R, tag=f"w3_{dx}_{kt}", name=f"w3_{dx}_{kt}")
                          for kt, (ks, ksz) in enumerate(M482)] for dx in range(2)]
                for dx in range(2):
                    for kt, (ks, ksz) in enumerate(M482):
                        nc.scalar.dma_start(w3_sb[dx][kt][:],
                                            w3T.ap()[dx, ks:ks + ksz, :])
                w4_sb = [[cp.tile([ksz, 100], F32, tag=f"w4_{dx}_{kt}", name=f"w4_{dx}_{kt}")
                          for kt, (ks, ksz) in enumerate(M241)] for dx in range(2)]
                for dx in range(2):
                    for kt, (ks, ksz) in enumerate(M241):
                        nc.scalar.dma_start(w4_sb[dx][kt][:],
                                            w4T.ap()[dx, ks:ks + ksz, :])
                fcnn_sb = cp.tile([100, 23, 100], F32, tag="fcnn", name="fcnn")
                nc.scalar.dma_start(
                    fcnn_sb[:],
                    fccnnT.ap().rearrange("(w c) o -> c w o", c=100))
                wih_sb = load("wih", wihT, eng=nc.scalar)
                whh_sb = load("whh", whhT, eng=nc.scalar)
                bih_sb = load("bih", lstmb, eng=nc.scalar)
                bhh_sb = load("bhh", lstmb2, eng=nc.scalar)
                fco_sb = load("fco", fcoT, eng=nc.scalar)
                fob_sb = load("fob", fcob, eng=nc.scalar)

                if stage < 2:
                    return
                # ---- W1sum: shard-reduce cnn1_w over emb channels + AllReduce
                # lhsT row order: 0..9 gated (k,dx); 10,11 W1sum dx; 12,13 sca dx
                lhsT = cp.tile([14, 964], F32R, tag="lhsT", name="lhsT")
                nc.sync.dma_start(lhsT[0:10, :], w1colsT.ap()[0:10, :])
                nc.sync.dma_start(lhsT[12:14, :], w1colsT.ap()[10:12, :])
                w1part = [wp.tile([msz, 2], F32, tag=f"w1p_{mt}", name=f"w1p_{mt}")
                          for mt, (ms, msz) in enumerate(M964)]
                w1p_b = dp.tile([964, 2], F32, tag="w1p_b", name="w1p_b")
                w1r_b = dp.tile([964, 2], F32, tag="w1r_b", name="w1r_b")
                for mt, (ms, msz) in enumerate(M964):
                    v = w1s_sb[mt][:].rearrange("m (c two) -> m c two", two=2)
                    for dx in range(2):
                        nc.vector.tensor_reduce(
                            w1part[mt][:, dx:dx + 1], v[:, :, dx],
                            axis=mybir.AxisListType.X, op=ALU.add)
                    nc.sync.dma_start(w1p_b[ms:ms + msz, :], w1part[mt][:])
                nc.gpsimd.collective_compute(
                    "AllReduce", ALU.add,
                    replica_groups=[list(range(N_CORES))],
                    ins=[w1p_b.opt()], outs=[w1r_b.opt()])
                for mt, (ms, msz) in enumerate(M964):
                    w1sum_t = wp.tile([msz, 2], F32, tag=f"w1sum_{mt}", name=f"w1sum_{mt}")
                    nc.sync.dma_start(w1sum_t[:], w1r_b[ms:ms + msz, :])
                    tp = pcb.tile([2, msz], F32, tag="sps", name="sps")
                    nc.tensor.transpose(tp[:], w1sum_t[:], ident_sb[0:msz, 0:msz])
                    w1t_sb = wp.tile([2, msz], F32R, tag="w1t", name="w1t", bufs=2)
                    nc.vector.tensor_copy(w1t_sb[:], tp[:])
                    nc.sync.dma_start(lhsT[10:12, ms:ms + msz], w1t_sb[:])

                if stage < 3:
                    return
                # ---- scalar branch: conv_con 4x4 stride2 + relu + pool ------
                acc = wp.tile([63, F, 63], F32, tag="acc", name="acc")
                for r in range(4):
                    rar = row_tiles[r][:, :, :].rearrange(
                        "p f (w two) -> p f w two", two=2)
                    for cc in range(4):
                        u, vv = cc // 2, cc % 2
                        view = rar[:, :, u:u + 63, vv]
                        widx = 4 * r + cc
                        if r == 0 and cc == 0:
                            nc.vector.tensor_scalar(
                                out=acc[:], in0=view,
                                scalar1=ccw_sb[0:63, widx:widx + 1],
                                scalar2=None, op0=ALU.mult)
                        else:
                            nc.vector.scalar_tensor_tensor(
                                out=acc[:], in0=view,
                                scalar=ccw_sb[0:63, widx:widx + 1],
                                in1=acc[:], op0=ALU.mult, op1=ALU.add)
                # cp_t free layout is (q, f) so the flatten-shuffle DMA below
                # reads each partition row contiguously
                cp_t = wp.tile([63, 31, F], F32, tag="cpool", name="cpool")
                av = acc[:, :, 0:62].rearrange("p f (w two) -> p f w two", two=2)
                nc.vector.tensor_tensor(out=cp_t[:].rearrange("p q f -> p f q"),
                                        in0=av[:, :, :, 0],
                                        in1=av[:, :, :, 1], op=ALU.max)
                nc.vector.tensor_scalar(out=cp_t[:], in0=cp_t[:],
                                        scalar1=ccb_sb[0:63, 0:1], op0=ALU.add,
                                        scalar2=0.0, op1=ALU.max)
                # row pool + flatten shuffle into fc_con rhs K-tiles
                rhsfc = [wp.tile([ksz, F], F32, tag=f"rhsfc_{t_}", name=f"rhsfc_{t_}")
                         for t_, (ks, ksz) in enumerate(K961)]
                for t_, (ks, ksz) in enumerate(K961):
                    np_rows = ksz // 31  # 4 or 3 p-rows in this tile
                    e2 = wp.tile([ksz, F], F32, tag=f"e2_{t_}", name=f"e2_{t_}")
                    o2 = wp.tile([ksz, F], F32, tag=f"o2_{t_}", name=f"o2_{t_}")
                    for pl in range(np_rows):
                        p = 4 * t_ + pl
                        for half, dst in ((0, e2), (1, o2)):
                            src = cp_t[2 * p + half: 2 * p + half + 1, :, :]
                            # [1 part, (q,f) contiguous] -> [31 part, F]
                            eng = nc.sync if half == 0 else nc.scalar
                            eng.dma_start(
                                dst[31 * pl:31 * pl + 31, :],
                                src.rearrange("one q f -> one (q f)"))
                    nc.vector.tensor_tensor(out=rhsfc[t_][:], in0=e2[:],
                                            in1=o2[:], op=ALU.max)
                # fc_con GEMM -> yT [200, F] as two [100, F] tiles
                yT = []
                for m2 in range(2):
                    yps = pcb.tile([100, F], F32, tag="yps", name="yps")
                    for t_, (ks, ksz) in enumerate(K961):
                        nc.tensor.matmul(
                            yps[:], fcc_sb[t_][:, 100 * m2:100 * m2 + 100],
                            rhsfc[t_][:],
                            start=(t_ == 0), stop=(t_ == len(K961) - 1))
                    yt = wp.tile([100, F], F32, tag=f"yT_{m2}", name=f"yT_{m2}")
                    nc.scalar.activation(yt[:], yps[:], AF.Identity,
                                         bias=fcb_sb[:, m2:m2 + 1])
                    yT.append(yt)
                y_sb = wp.tile([F, 200], F32, tag="y", name="y")
                for m2 in range(2):
                    tp = pcb.tile([F, 100], F32, tag="sps", name="sps")
                    nc.tensor.transpose(tp[:], yT[m2][:], ident_sb[0:100, 0:100])
                    nc.vector.tensor_copy(y_sb[:, 100 * m2:100 * m2 + 100], tp[:])

                if stage < 5:
                    return
                # ---- rhs assembly for the collapsed conv1 ------------------
                # rhs row order matches lhsT: 0..9 gated; 10,11 e0; 12,13 y
                rhs = wp.tile([14, F, 199], F32R, tag="rhs", name="rhs")
                for dx in range(2):
                    for f in range(F):
                        eng = nc.sync if f % 2 == 0 else nc.scalar
                        eng.dma_start(rhs[10 + dx:11 + dx, f, :],
                                      emb01.ap()[0:1, dx:dx + 199].bitcast(F32R))
                d_t = wp.tile([1, 200], F32, tag="d", name="d")
                nc.vector.tensor_tensor(out=d_t[:], in0=emb1_sb[0:1, :],
                                        in1=emb_sb[0:1, :], op=ALU.subtract)
                dtile = wp.tile([10, 199], F32, tag="dtile", name="dtile")
                for k in range(5):
                    for dx in range(2):
                        nc.sync.dma_start(dtile[2 * k + dx:2 * k + dx + 1, :],
                                          d_t[0:1, dx:dx + 199])
                flip = wp.tile([10, F], F32, tag="flip", name="flip")
                nc.vector.tensor_scalar(out=flip[:], in0=v0_sb[:], scalar1=1.0,
                                        scalar2=None, op0=ALU.is_equal)
                for f in range(F):
                    nc.vector.tensor_scalar(out=rhs[0:10, f, :], in0=dtile[:],
                                            scalar1=flip[:, f:f + 1],
                                            scalar2=None, op0=ALU.mult)
                for dx in range(2):
                    nc.sync.dma_start(rhs[12 + dx:13 + dx, :, :],
                                      y_sb[:, dx:dx + 199].bitcast(F32R))

                if stage < 6:
                    return
                # ---- conv1 (collapsed) + pool -> pooled1 -------------------
                pooled1 = [cp.tile([msz, F, 99], F32R, tag=f"p1_{mt}", name=f"p1_{mt}")
                           for mt, (ms, msz) in enumerate(M964)]
                for mt, (ms, msz) in enumerate(M964):
                    for fg in range(0, F, 2):
                        nf = min(2, F - fg)
                        ps = pca.tile([msz, nf, 199], F32, tag="cps", name="cps")
                        nc.tensor.matmul(ps[:], _r(lhsT[:, ms:ms + msz]),
                                         _r(rhs[:, fg:fg + nf, :]),
                                         start=True, stop=True)
                        # pool+bias+relu on DVE (only one psum operand per op)
                        ev = ps[:, :, 0:198].rearrange(
                            "m f (w two) -> m f w two", two=2)
                        dst = pooled1[mt][:, fg:fg + nf, :]
                        todd1 = wp.tile([msz, nf, 99], F32, tag="c1tmp",
                                        name="c1tmp", bufs=2)
                        nc.vector.tensor_copy(todd1[:], ev[:, :, :, 1])
                        nc.vector.tensor_tensor(out=dst, in0=ev[:, :, :, 0],
                                                in1=todd1[:], op=ALU.max)
                        nc.vector.tensor_scalar(out=dst, in0=dst,
                                                scalar1=c1b_sb[0:msz, mt:mt + 1],
                                                op0=ALU.add, scalar2=0.0,
                                                op1=ALU.max)

                if stage < 7:
                    return
                # ---- conv2 + pool -> pooled2 -------------------------------
                pooled2 = [cp.tile([msz, F, 49], F32R, tag=f"p2_{mt}", name=f"p2_{mt}")
                           for mt, (ms, msz) in enumerate(M482)]
                for mt, (ms, msz) in enumerate(M482):
                    for fg in range(0, F, 4):
                        nf = min(4, F - fg)
                        ps = pca.tile([msz, nf, 98], F32, tag="cps", name="cps")
                        nmm = len(M964) * 2
                        i = 0
                        for kt, (ks, ksz) in enumerate(M964):
                            for dx in range(2):
                                nc.tensor.matmul(
                                    ps[:], _r(w2_sb[dx][kt][:, ms:ms + msz]),
                                    _r(pooled1[kt][:, fg:fg + nf, dx:dx + 98]),
                                    start=(i == 0), stop=(i == nmm - 1))
                                i += 1
                        ev = ps[:, :, 0:98].rearrange(
                            "m f (w two) -> m f w two", two=2)
                        dst = pooled2[mt][:, fg:fg + nf, :]
                        todd = wp.tile([msz, nf, 49], F32, tag="c2tmp",
                                       name="c2tmp", bufs=2)
                        nc.vector.tensor_copy(todd[:], ev[:, :, :, 1])
                        nc.vector.tensor_tensor(out=dst, in0=ev[:, :, :, 0],
                                                in1=todd[:], op=ALU.max)
                        nc.vector.tensor_scalar(out=dst, in0=dst,
                                                scalar1=c2b_sb[0:msz, mt:mt + 1],
                                                op0=ALU.add, scalar2=0.0,
                                                op1=ALU.max)

                if stage < 8:
                    return
                # ---- conv3 + pool -> pooled3 -------------------------------
                pooled3 = [cp.tile([msz, F, 24], F32, tag=f"p3_{mt}", name=f"p3_{mt}")
                           for mt, (ms, msz) in enumerate(M241)]
                for mt, (ms, msz) in enumerate(M241):
                    ps = pca.tile([msz, F, 48], F32, tag="cps", name="cps")
                    nmm = len(M482) * 2
                    i = 0
                    for kt, (ks, ksz) in enumerate(M482):
                        for dx in range(2):
                            nc.tensor.matmul(
                                ps[:], _r(w3_sb[dx][kt][:, ms:ms + msz]),
                                _r(pooled2[kt][:, :, dx:dx + 48]),
                                start=(i == 0), stop=(i == nmm - 1))
                            i += 1
                    ev = ps[:, :, 0:48].rearrange("m f (w two) -> m f w two", two=2)
                    dst = pooled3[mt][:]
                    todd3 = wp.tile([msz, F, 24], F32, tag="c3tmp",
                                    name="c3tmp", bufs=2)
                    nc.vector.tensor_copy(todd3[:], ev[:, :, :, 1])
                    nc.vector.tensor_tensor(out=dst, in0=ev[:, :, :, 0],
                                            in1=todd3[:], op=ALU.max)
                    nc.vector.tensor_scalar(out=dst, in0=dst,
                                            scalar1=c3b_sb[0:msz, mt:mt + 1],
                                            op0=ALU.add, scalar2=0.0, op1=ALU.max)

                if stage < 8:
                    return
                # ---- conv4 + relu -> conv4o [100, F, 23] -------------------
                conv4o = wp.tile([100, F, 23], F32, tag="c4o", name="c4o")
                ps4 = pca.tile([100, F, 23], F32, tag="cps", name="cps")
                i = 0
                for kt, (ks, ksz) in enumerate(M241):
                    for dx in range(2):
                        nc.tensor.matmul(ps4[:], w4_sb[dx][kt][:],
                                         pooled3[kt][:, :, dx:dx + 23],
                                         start=(i == 0), stop=(i == 3))
                        i += 1
                nc.scalar.activation(conv4o[:], ps4[:], AF.Relu,
                                     bias=c4b_sb[:, 0:1])

                if stage < 8:
                    return
                # ---- fc_cnn -> seqT_j [100, F]; AllGather -> seqT [100, BT]
                agin = dp.tile([100, F], F32, tag="agin", name="agin")
                agout = dp.tile([N_CORES * 100, F], F32, tag="agout",
                                addr_space="Shared")
                ps5 = pcb.tile([100, F], F32, tag="yps", name="yps")
                for w in range(23):
                    nc.tensor.matmul(ps5[:], fcnn_sb[:, w, :], conv4o[:, :, w],
                                     start=(w == 0), stop=(w == 22))
                seqj = wp.tile([100, F], F32, tag="seqj", name="seqj")
                nc.scalar.activation(seqj[:], ps5[:], AF.Identity,
                                     bias=fnb_sb[:, 0:1])
                nc.sync.dma_start(agin[:], seqj[:])
                nc.gpsimd.collective_compute(
                    "AllGather", ALU.bypass,
                    replica_groups=[list(range(N_CORES))],
                    ins=[agin.opt()], outs=[agout.opt()])
                seqT = wp.tile([100, BT], F32, tag="seqT", name="seqT")
                for j in range(N_CORES):
                    nc.sync.dma_start(seqT[:, F * j:F * j + F],
                                      agout[100 * j:100 * j + 100, :])

                if stage < 9:
                    return
                # ---- LSTM (replicated) -------------------------------------
                badd = wp.tile([100, 4], F32, tag="badd", name="badd")
                nc.vector.tensor_tensor(out=badd[:], in0=bih_sb[:], in1=bhh_sb[:],
                                        op=ALU.add)
                xpall = wp.tile([100, T, 4 * GW], F32, tag="xpall", name="xpall")
                for g in range(4):
                    xps = pcb.tile([100, BT], F32, tag="sps", name="sps")
                    nc.tensor.matmul(xps[:], wih_sb[:, 100 * g:100 * g + 100],
                                     seqT[:], start=True, stop=True)
                    xp = wp.tile([100, B, T], F32, tag="xp", name="xp")
                    nc.scalar.activation(xp[:], xps[:].rearrange("p (b t) -> p b t", b=B),
                                         AF.Identity, bias=badd[:, g:g + 1])
                    nc.vector.tensor_copy(xpall[:, :, GW * g:GW * g + GW],
                                          xp[:].rearrange("p b t -> p t b"))
                h_t = wp.tile([100, GW], F32, tag="h", name="h")
                c_t = wp.tile([100, GW], F32, tag="c", name="c")
                nc.vector.memset(h_t[:], 0.0)
                nc.vector.memset(c_t[:], 0.0)
                for t_ in range(n_steps):
                    gps = pcb.tile([100, 4 * GW], F32, tag="sps", name="sps")
                    for g in range(4):
                        nc.tensor.matmul(gps[:, GW * g:GW * g + GW],
                                         whh_sb[:, 100 * g:100 * g + 100],
                                         h_t[:], start=True, stop=True)
                    gsb = wp.tile([100, 4 * GW], F32, tag="gsb", name="gsb")
                    nc.vector.tensor_tensor(out=gsb[:], in0=gps[:],
                                            in1=xpall[:, t_, :], op=ALU.add)
                    acts = wp.tile([100, 4 * GW], F32, tag="acts", name="acts")
                    nc.scalar.activation(acts[:, 0:3 * GW], gsb[:, 0:3 * GW],
                                         AF.Sigmoid)
                    nc.scalar.activation(acts[:, 3 * GW:4 * GW],
                                         gsb[:, 3 * GW:4 * GW], AF.Tanh)
                    t1 = wp.tile([100, GW], F32, tag="t1", name="t1")
                    t2 = wp.tile([100, GW], F32, tag="t2", name="t2")
                    nc.vector.tensor_tensor(out=t1[:], in0=acts[:, 0:GW],
                                            in1=acts[:, 3 * GW:4 * GW],
                                            op=ALU.mult)
                    nc.vector.tensor_tensor(out=t2[:], in0=acts[:, GW:2 * GW],
                                            in1=c_t[:], op=ALU.mult)
                    nc.vector.tensor_tensor(out=c_t[:], in0=t1[:], in1=t2[:],
                                            op=ALU.add)
                    th = wp.tile([100, GW], F32, tag="th", name="th")
                    nc.scalar.activation(th[:], c_t[:], AF.Tanh)
                    nc.vector.tensor_tensor(out=h_t[:], in0=acts[:, 2 * GW:3 * GW],
                                            in1=th[:], op=ALU.mult)
                ops = pcb.tile([B, 2], F32, tag="sps", name="sps")
                nc.tensor.matmul(ops[:], h_t[:], fco_sb[:], start=True, stop=True)
                out_sb = wp.tile([B, 2], F32, tag="out", name="out")
                nc.scalar.activation(out_sb[:], ops[:], AF.Identity,
                                     bias=fob_sb[0:B, 0:1])
                nc.sync.dma_start(out.ap(), out_sb[:])

    _emit()
    nc.compile()
    return nc


_CACHE = {}


def _get_nc(B):
    if B not in _CACHE:
        _CACHE[B] = _build(B)
    return _CACHE[B]


def _prep_inputs(B, **inp):
    """Host-side sharding/layout prep. Pure slicing/transpose/reshape."""
    f32 = np.float32
    T_ = T
    replay = np.asarray(inp["replay_tensor"], f32)[:B]
    BT = B * T_
    F = BT // N_CORES
    frames = replay.reshape(BT, 8, 128, 128)
    scal_all = np.ascontiguousarray(frames[:, 0])              # [BT,128,128]
    v0_all = np.ascontiguousarray(frames[:, CAT_FEATS, 0, 0])  # [BT,5]

    emb = np.asarray(inp["emb_table"], f32)
    w1 = np.asarray(inp["cnn1_w"], f32)[:, :, 0, :]            # [964,1928,2]
    w1colsT = np.stack(
        [w1[:, c, d] for c in CKZ for d in (0, 1)]
        + [w1[:, 0, d] for d in (0, 1)]).astype(f32)           # [12,964]

    def padbias(b, mts, cols):
        b = np.asarray(b, f32)
        o = np.zeros((128, cols), f32)
        for i, (s, sz) in enumerate(mts):
            o[:sz, i] = b[s:s + sz]
        return o

    ccw_bc = np.broadcast_to(
        np.asarray(inp["conv_con_w"], f32).reshape(16)[None, :],
        (128, 16)).copy()
    ccb_bc = np.full((128, 1), np.asarray(inp["conv_con_b"], f32).reshape(1)[0],
                     f32)
    fcconT = np.ascontiguousarray(np.asarray(inp["fc_con_w"], f32).T)  # [961,200]
    fcconb = np.ascontiguousarray(
        np.asarray(inp["fc_con_b"], f32).reshape(2, 100).T)            # [100,2]
    w2T = np.ascontiguousarray(
        np.asarray(inp["cnn2_w"], f32)[:, :, 0, :].transpose(2, 1, 0))  # [2,964,482]
    w3T = np.ascontiguousarray(
        np.asarray(inp["cnn3_w"], f32)[:, :, 0, :].transpose(2, 1, 0))
    w4T = np.ascontiguousarray(
        np.asarray(inp["cnn4_w"], f32)[:, :, 0, :].transpose(2, 1, 0))
    fccnnT = np.ascontiguousarray(
        np.asarray(inp["fc_cnn_w"], f32).reshape(100, 100, 23)
        .transpose(2, 1, 0).reshape(2300, 100))                # rows 100w+c
    perm = np.r_[0:100, 100:200, 300:400, 200:300]             # (i,f,o,g)
    wihT = np.ascontiguousarray(np.asarray(inp["lstm_w_ih"], f32)[perm].T)
    whhT = np.ascontiguousarray(np.asarray(inp["lstm_w_hh"], f32)[perm].T)
    bih = np.ascontiguousarray(
        np.asarray(inp["lstm_b_ih"], f32)[perm].reshape(4, 100).T)
    bhh = np.ascontiguousarray(
        np.asarray(inp["lstm_b_hh"], f32)[perm].reshape(4, 100).T)
    fcoT = np.ascontiguousarray(np.asarray(inp["fc_out_w"], f32).T)  # [100,2]
    fcob = np.asarray(inp["fc_out_b"], f32).reshape(2, 1)

    common = dict(
        w1colsT=w1colsT,
        emb01=np.ascontiguousarray(emb[0:2]),
        emb10=np.ascontiguousarray(emb[[1, 0]]),
        ccw=ccw_bc, ccb=ccb_bc,
        fcconT=fcconT, fcconb=fcconb,
        cnn1b=padbias(inp["cnn1_b"], _tiles(964), 8),
        w2T=w2T, cnn2b=padbias(inp["cnn2_b"], _tiles(482), 4),
        w3T=w3T, cnn3b=padbias(inp["cnn3_b"], _tiles(241), 2),
        w4T=w4T, cnn4b=np.asarray(inp["cnn4_b"], f32).reshape(100, 1),
        fccnnT=fccnnT,
        fccnnb=np.asarray(inp["fc_cnn_b"], f32).reshape(100, 1),
        wihT=wihT, whhT=whhT, lstmb=bih, lstmb2=bhh,
        fcoT=fcoT, fcob=fcob,
        ident=np.eye(128, dtype=f32),
    )

    in_maps = []
    for j in range(N_CORES):
        lo = 1 + CH_PER_CORE * j
        hi = min(lo + CH_PER_CORE, 1928)
        sl = w1[:, lo:hi, :]
        if hi - lo < CH_PER_CORE:
            sl = np.concatenate(
                [sl, np.zeros((964, CH_PER_CORE - (hi - lo), 2), f32)], axis=1)
        m = dict(common)
        m["scal"] = np.ascontiguousarray(scal_all[F * j:F * j + F])
        m["v0dup"] = np.ascontiguousarray(
            np.repeat(v0_all[F * j:F * j + F].T, 2, axis=0))   # [10,F]
        m["w1slice"] = np.ascontiguousarray(sl.reshape(964, 2 * CH_PER_CORE))
        in_maps.append(m)
    return in_maps


def kernel(**inputs):
    B = int(np.asarray(inputs.get("batch_size", 2)))
    if B <= 0:
        return np.zeros((0, 2), np.float32)
    nc = _get_nc(B)
    in_maps = _prep_inputs(B, **inputs)
    res = run_bass_kernel_spmd(nc, in_maps, list(range(N_CORES)))
    return np.asarray(res.results[0]["out"], np.float32)



# revision 19
# speedup vs baseline: 2.0142x; 1.5873x over previous
"""Trainium2 Bass kernel for nn_ConvLSTMModel_85418309583256.

Structure (exploiting the model's degenerate embedding branch):
  - replay values are in {0,1}; the categorical/embedding branch reads only
    spatial position (0,0) of each frame, so the [B,T,1927,200] "embedding"
    tensor is emb_table[0] everywhere except at most 5 known channels
    (z-channels 2,6,8,13,15) that flip to emb_table[1] when v0[k]==1.
    conv1 therefore collapses to a K=15 matmul per frame (10 gated rows,
    2 W1sum rows, a bias row, 2 scalar-branch rows), with W1sum and all
    conv biases folded on the host into the weight tensors.
  - data parallel over B*T frames: 8 frames per core on 8 cores; one
    AllGather reshards the per-frame seq vectors; the tiny LSTM
    (hidden 100) runs replicated on every core.
  - conv1/2/3 matmuls run in float32r (1 cycle/row at N>=256 vs 4 for
    fp32); pooling is relu(odd) on ACT + max on DVE, biases ride as
    an appended ones-row K contribution inside each matmul.
  - LSTM uses sigmoid(x)=0.5*(1+tanh(x/2)) so all four gates need ONE
    tanh, with the 0.5 factors folded into the weights; cell state is
    kept as C=2c and hidden as H=2h so each step is 4 matmuls (PSUM
    preloaded with the x-projection), one gate tanh, three
    scalar_tensor_tensor ops, one tanh, one scalar_tensor_tensor.
"""

import sys
for _p in ("/opt/trn_rl_repo", "/root/.axon_site/_ro/trn_rl_repo"):
    if _p not in sys.path:
        sys.path.append(_p)

import numpy as np
import concourse.bass as bass
import concourse.bacc as bacc
import concourse.mybir as mybir
from concourse import tile
from concourse.bass_utils import run_bass_kernel_spmd

F32 = mybir.dt.float32
F32R = mybir.dt.float32r
AF = mybir.ActivationFunctionType
ALU = mybir.AluOpType

N_CORES = 8
T = 32
# feature indices (of the 8 input features) used by the categorical branch
CAT_FEATS = [1, 3, 5, 6, 7]
# z-space channels (within the 1928-channel CNN input) that flip to e1
CKZ = [2, 6, 8, 13, 15]

# consts pack column offsets
C_CCW = 0      # [128,16] conv_con taps (broadcast)
C_CCB = 16     # [128,1]  conv_con bias (broadcast)
C_V0 = 17      # [10,F]   v0 values duplicated x2
C_DT = 25      # [10,199] (e1-e0)[dx:dx+199] rows
C_FOB = 224    # [2,1]    fc_out bias
NCC = 225


def _tiles(total, step=128):
    return [(s, min(step, total - s)) for s in range(0, total, step)]


def _build(B, n_steps=None):
    if n_steps is None:
        n_steps = T
    BT = B * T
    assert BT % N_CORES == 0
    F = BT // N_CORES
    GW = B

    nc = bacc.Bacc("TRN2", target_bir_lowering=False, debug=False,
                   num_devices=N_CORES)

    def din(name, shape, dt=F32):
        return nc.dram_tensor(name, list(shape), dt, kind="ExternalInput")

    scal4 = din("scal4", (64, 2, F, 128))
    consts = din("consts", (128, NCC))
    lhsT_h = din("lhsT_h", (15, 964), F32R)
    e0dup3 = din("e0dup3", (3, F * 199), F32R)   # rows: e0 dx0, e0 dx1, ones
    fcc2 = din("fcc2", (124, 8, 200))
    w2p = din("w2p", (128, 8, 2, 482), F32R)
    w3p = din("w3p", (128, 4, 2, 241), F32R)
    w4p = din("w4p", (128, 2, 2, 100))
    fcnn2 = din("fcnn2", (100, 24, 100))
    lstmpk = din("lstmpk", (100, 806))           # wih 0:400, whh 400:800, b 800:804, fco 804:806

    out = nc.dram_tensor("out", [B, 2], F32, kind="ExternalOutput")

    M964 = _tiles(964)
    M482 = _tiles(482)
    M241 = _tiles(241)

    with tile.TileContext(nc) as tc:
        with (
            tc.tile_pool(name="cst", bufs=1) as cp,
            tc.tile_pool(name="wrk", bufs=1) as wp,
            tc.tile_pool(name="cps", bufs=3, space="PSUM") as pca,
            tc.tile_pool(name="sps", bufs=2, space="PSUM") as pcb,
            tc.tile_pool(name="lps", bufs=3, space="PSUM") as pcl,
            tc.tile_pool(name="dram", bufs=1, space="DRAM") as dp,
        ):
            # ---- loads --------------------------------------------------
            rows_sb = cp.tile([64, 2, F, 128], F32, tag="rows", name="rows")
            nc.sync.dma_start(rows_sb[:], scal4.ap())
            rowsb_sb = cp.tile([63, 2, F, 128], F32, tag="rowsb", name="rowsb")
            nc.sync.dma_start(rowsb_sb[:], scal4.ap()[1:64])
            lhsT_sb = cp.tile([15, 964], F32R, tag="lhsT", name="lhsT")
            nc.sync.dma_start(lhsT_sb[:], lhsT_h.ap())
            cst_sb = cp.tile([128, NCC], F32, tag="cst", name="cst")
            nc.scalar.dma_start(cst_sb[:], consts.ap())
            fcc_sb = cp.tile([124, 8, 200], F32, tag="fcc", name="fcc")
            nc.scalar.dma_start(fcc_sb[:], fcc2.ap())
            fcnn_sb = cp.tile([100, 24, 100], F32, tag="fcnn", name="fcnn")
            nc.scalar.dma_start(fcnn_sb[:], fcnn2.ap())
            w2_sb = cp.tile([128, 8, 2, 482], F32R, tag="w2", name="w2")
            nc.gpsimd.dma_start(w2_sb[:, 0:4, :, :], w2p.ap()[:, 0:4, :, :])
            nc.gpsimd.dma_start(w2_sb[:, 4:8, :, :], w2p.ap()[:, 4:8, :, :])
            w3_sb = cp.tile([128, 4, 2, 241], F32R, tag="w3", name="w3")
            nc.gpsimd.dma_start(w3_sb[:], w3p.ap())
            w4_sb = cp.tile([128, 2, 2, 100], F32, tag="w4", name="w4")
            nc.gpsimd.dma_start(w4_sb[:], w4p.ap())
            lst_sb = cp.tile([100, 806], F32, tag="lst", name="lst")
            nc.gpsimd.dma_start(lst_sb[:], lstmpk.ap())

            # ---- conv_con (4x4 stride-2 conv on DVE+Pool) ---------------
            def rt(r):
                return (rows_sb if r < 2 else rowsb_sb)[0:63, r % 2]

            acc = wp.tile([63, F, 63], F32, tag="acc", name="acc")
            for taps, eng, acc_ in ((list(range(16)), nc.vector, acc),):
                for n, widx in enumerate(taps):
                    r, cc = widx // 4, widx % 4
                    rar = rt(r)[:, :, :].rearrange(
                        "p f (w two) -> p f w two", two=2)
                    view = rar[:, :, cc // 2:cc // 2 + 63, cc % 2]
                    if n == 0:
                        eng.tensor_scalar(
                            out=acc_[:], in0=view,
                            scalar1=cst_sb[0:63, C_CCW + widx:C_CCW + widx + 1],
                            scalar2=None, op0=ALU.mult)
                    else:
                        eng.scalar_tensor_tensor(
                            out=acc_[:], in0=view,
                            scalar=cst_sb[0:63, C_CCW + widx:C_CCW + widx + 1],
                            in1=acc_[:], op0=ALU.mult, op1=ALU.add)
            # column pool -> cp_t [63, 31(q), F]
            cp_t = wp.tile([63, 31, F], F32, tag="cpool", name="cpool")
            av = acc[:, :, 0:62].rearrange("p f (q two) -> p q f two", two=2)
            nc.vector.tensor_tensor(out=cp_t[:], in0=av[:, :, :, 0],
                                    in1=av[:, :, :, 1], op=ALU.max)
            nc.vector.tensor_scalar(out=cp_t[:], in0=cp_t[:],
                                    scalar1=cst_sb[0:63, C_CCB:C_CCB + 1],
                                    op0=ALU.add, scalar2=0.0, op1=ALU.max)
            # row pool via DRAM-roundtrip deinterleave; rows 62/63 are set
            # to exactly 1.0 so chunk-7 pad partitions carry the fc_con
            # bias row (fcc2 row 961) and zero-padded rows beyond it.
            scr = dp.tile([64, 31, F], F32, tag="scr", name="scr")
            nc.sync.dma_start(scr[0:63, :, :], cp_t[:])
            nc.sync.dma_start(scr[62:64, :, :],
                              e0dup3.ap()[2:3, 0:2 * 31 * F].bitcast(F32))
            e2t = wp.tile([124, 8, F], F32, tag="e2t", name="e2t")
            o2t = wp.tile([124, 8, F], F32, tag="o2t", name="o2t")
            sv = scr[:].rearrange("(c eight) q f -> eight q c f", eight=8)
            for pl in range(4):
                nc.sync.dma_start(e2t[31 * pl:31 * pl + 31, :, :],
                                  sv[2 * pl, :, :, :])
                nc.scalar.dma_start(o2t[31 * pl:31 * pl + 31, :, :],
                                    sv[2 * pl + 1, :, :, :])
            rhsfc = wp.tile([124, 8, F], F32, tag="rhsfc", name="rhsfc")
            nc.vector.tensor_tensor(out=rhsfc[:], in0=e2t[:], in1=o2t[:],
                                    op=ALU.max)

            # ---- fc_con GEMM: y [F, 200] --------------------------------
            yps = pcb.tile([F, 200], F32, tag="sps", name="yps")
            for c in range(8):
                nc.tensor.matmul(yps[:], rhsfc[:, c, :], fcc_sb[:, c, :],
                                 start=(c == 0), stop=(c == 7))
            y_sb = wp.tile([F, 200], F32, tag="y", name="y")
            nc.scalar.activation(y_sb[:], yps[:], AF.Identity)

            # ---- rhs assembly [15, F, 199] ------------------------------
            # rows 0..9 gated; 10,11 e0; 12 ones; 13,14 y
            rhs = wp.tile([15, F, 199], F32R, tag="rhs", name="rhs")
            flip = wp.tile([10, F], F32, tag="flip", name="flip")
            nc.vector.tensor_scalar(out=flip[:],
                                    in0=cst_sb[0:10, C_V0:C_V0 + F],
                                    scalar1=1.0, scalar2=None,
                                    op0=ALU.is_equal)
            dt_bc = cst_sb[0:10, C_DT:C_DT + 199].rearrange(
                "p (one w) -> p one w", one=1).broadcast_to([10, F, 199])
            fl_bc = flip[:].rearrange(
                "p (f one) -> p f one", one=1).broadcast_to([10, F, 199])
            nc.vector.tensor_tensor(out=rhs[0:10, :, :], in0=dt_bc,
                                    in1=fl_bc, op=ALU.mult)
            nc.sync.dma_start(rhs[10:13, :, :], e0dup3.ap())
            for dx in range(2):
                nc.sync.dma_start(rhs[13 + dx:14 + dx, :, :],
                                  y_sb[:, dx:dx + 199].bitcast(F32R))

            ones_sb = wp.tile([128, 1], F32, tag="ones", name="ones")
            nc.vector.memset(ones_sb[:], 1.0)

            def fill_ones_r(dst, np_, f_, w_):
                nc.vector.tensor_scalar(
                    out=dst, scalar1=1.0, op0=ALU.mult, scalar2=None,
                    in0=ones_sb[0:np_, 0:1].rearrange(
                        "p (a b) -> p a b", a=1).broadcast_to([np_, f_, w_]))

            # ---- conv1 (collapsed, K=15) + pool -> pooled1 --------------
            pooled1 = [cp.tile([msz + (1 if mt == len(M964) - 1 else 0), F, 99],
                               F32R, tag=f"p1_{mt}", name=f"p1_{mt}")
                       for mt, (ms, msz) in enumerate(M964)]
            fill_ones_r(pooled1[-1][0:69, :, :], 69, F, 99)
            for mt, (ms, msz) in enumerate(M964):
                for fg in range(0, F, 2):
                    nf = min(2, F - fg)
                    ps = pca.tile([msz, nf, 199], F32, tag="cps", name="cps")
                    nc.tensor.matmul(ps[:], lhsT_sb[:, ms:ms + msz],
                                     rhs[:, fg:fg + nf, :],
                                     start=True, stop=True)
                    ev = ps[:, :, 0:198].rearrange(
                        "m f (w two) -> m f w two", two=2)
                    orl = wp.tile([msz, nf, 99], F32, tag="orl1",
                                  name="orl1", bufs=2)
                    nc.scalar.activation(orl[:], ev[:, :, :, 1], AF.Relu)
                    nc.vector.tensor_tensor(
                        out=pooled1[mt][0:msz, fg:fg + nf, :],
                        in0=ev[:, :, :, 0], in1=orl[:], op=ALU.max)

            # ---- conv2 + pool -> pooled2 --------------------------------
            pooled2 = [cp.tile([msz + (1 if mt == len(M482) - 1 else 0), F, 49],
                               F32R, tag=f"p2_{mt}", name=f"p2_{mt}")
                       for mt, (ms, msz) in enumerate(M482)]
            fill_ones_r(pooled2[-1][0:99, :, :], 99, F, 49)
            for mt, (ms, msz) in enumerate(M482):
                for fg in range(0, F, 4):
                    nf = min(4, F - fg)
                    ps = pca.tile([msz, nf, 98], F32, tag="cps", name="cps")
                    i = 0
                    for kt, (ks, ksz) in enumerate(M964):
                        kk = ksz + (1 if kt == len(M964) - 1 else 0)
                        for dx in range(2):
                            nc.tensor.matmul(
                                ps[:], w2_sb[0:kk, kt, dx, ms:ms + msz],
                                pooled1[kt][0:kk, fg:fg + nf, dx:dx + 98],
                                start=(i == 0), stop=(i == 15))
                            i += 1
                    ev = ps[:, :, 0:98].rearrange(
                        "m f (w two) -> m f w two", two=2)
                    orl = wp.tile([msz, nf, 49], F32, tag="orl2",
                                  name="orl2", bufs=2)
                    nc.scalar.activation(orl[:], ev[:, :, :, 1], AF.Relu)
                    nc.vector.tensor_tensor(
                        out=pooled2[mt][0:msz, fg:fg + nf, :],
                        in0=ev[:, :, :, 0], in1=orl[:], op=ALU.max)

            # ---- conv3 + pool -> pooled3 --------------------------------
            pooled3 = [cp.tile([msz + (1 if mt == len(M241) - 1 else 0), F, 24],
                               F32, tag=f"p3_{mt}", name=f"p3_{mt}")
                       for mt, (ms, msz) in enumerate(M241)]
            nc.vector.memset(pooled3[-1][0:114, :, :], 1.0)
            for mt, (ms, msz) in enumerate(M241):
                ps = pca.tile([msz, F, 48], F32, tag="cps", name="cps")
                i = 0
                for kt, (ks, ksz) in enumerate(M482):
                    kk = ksz + (1 if kt == len(M482) - 1 else 0)
                    for dx in range(2):
                        nc.tensor.matmul(
                            ps[:], w3_sb[0:kk, kt, dx, ms:ms + msz],
                            pooled2[kt][0:kk, :, dx:dx + 48],
                            start=(i == 0), stop=(i == 7))
                        i += 1
                ev = ps[:, :, 0:48].rearrange("m f (w two) -> m f w two", two=2)
                orl = wp.tile([msz, F, 24], F32, tag="orl3", name="orl3",
                              bufs=2)
                nc.scalar.activation(orl[:], ev[:, :, :, 1], AF.Relu)
                nc.vector.tensor_tensor(
                    out=pooled3[mt][0:msz, :, :],
                    in0=ev[:, :, :, 0], in1=orl[:], op=ALU.max)

            # ---- conv4 + relu -> conv4o [100, F, 23] --------------------
            conv4o = wp.tile([100, F, 23], F32, tag="c4o", name="c4o")
            ps4 = pca.tile([100, F, 23], F32, tag="cps", name="cps")
            i = 0
            for kt, (ks, ksz) in enumerate(M241):
                kk = ksz + (1 if kt == len(M241) - 1 else 0)
                for dx in range(2):
                    nc.tensor.matmul(ps4[:], w4_sb[0:kk, kt, dx, :],
                                     pooled3[kt][0:kk, :, dx:dx + 23],
                                     start=(i == 0), stop=(i == 3))
                    i += 1
            nc.scalar.activation(conv4o[:], ps4[:], AF.Relu)

            # ---- fc_cnn -> seqj [100, F]; AllGather -> seqT [100, BT] ---
            ones1f = wp.tile([1, F], F32, tag="ones1f", name="ones1f")
            nc.vector.memset(ones1f[:], 1.0)
            agin = dp.tile([100, F], F32, tag="agin", name="agin")
            agout = dp.tile([N_CORES * 100, F], F32, tag="agout",
                            addr_space="Shared")
            ps5 = pcb.tile([100, F], F32, tag="sps", name="ps5")
            for w in range(23):
                nc.tensor.matmul(ps5[:], fcnn_sb[:, w, :], conv4o[:, :, w],
                                 start=(w == 0), stop=False)
            nc.tensor.matmul(ps5[:], fcnn_sb[0:1, 23, :], ones1f[:],
                             start=False, stop=True)
            seqj = wp.tile([100, F], F32, tag="seqj", name="seqj")
            nc.scalar.activation(seqj[:], ps5[:], AF.Identity)
            nc.sync.dma_start(agin[:], seqj[:])
            nc.gpsimd.collective_compute(
                "AllGather", ALU.bypass,
                replica_groups=[list(range(N_CORES))],
                ins=[agin.opt()], outs=[agout.opt()])
            seqT = wp.tile([100, BT], F32, tag="seqT", name="seqT")
            for j in range(N_CORES):
                nc.sync.dma_start(seqT[:, F * j:F * j + F],
                                  agout[100 * j:100 * j + 100, :])

            # ---- LSTM x-projections: xpall [100, T, 4*GW] ---------------
            wih = lst_sb[:, 0:400]
            whh = lst_sb[:, 400:800]
            lstb = lst_sb[:, 800:804]
            fco = lst_sb[:, 804:806]
            xpall = wp.tile([100, T, 4 * GW], F32, tag="xpall", name="xpall")
            for g in range(4):
                xps = pcb.tile([100, BT], F32, tag="sps", name="xps")
                nc.tensor.matmul(xps[:], wih[:, 100 * g:100 * g + 100],
                                 seqT[:], start=True, stop=True)
                nc.scalar.activation(
                    xpall[:, :, GW * g:GW * g + GW],
                    xps[:].rearrange("p (b t) -> p t b", b=B),
                    AF.Identity, bias=lstb[:, g:g + 1])

            # ---- LSTM recurrence (C=2c, H=2h, tanh-only gates) ----------
            h_t = wp.tile([100, GW], F32, tag="h", name="h")
            c_t = wp.tile([100, GW], F32, tag="c", name="c")
            t1 = wp.tile([100, GW], F32, tag="t1", name="t1")
            t2 = wp.tile([100, GW], F32, tag="t2", name="t2")
            nc.vector.memset(h_t[:], 0.0)
            nc.vector.memset(c_t[:], 0.0)
            sl_i = slice(0, GW)
            sl_f = slice(GW, 2 * GW)
            sl_o = slice(2 * GW, 3 * GW)
            sl_g = slice(3 * GW, 4 * GW)
            for t_ in range(n_steps):
                gps = pcl.tile([100, 4 * GW], F32, tag="gps", name="gps")
                nc.scalar.activation(gps[:], xpall[:, t_, :], AF.Identity)
                for g in range(4):
                    nc.tensor.matmul(gps[:, GW * g:GW * g + GW],
                                     whh[:, 100 * g:100 * g + 100], h_t[:],
                                     start=False, stop=True,
                                     skip_group_check=True)
                acts = wp.tile([100, 4 * GW], F32, tag="acts", name="acts")
                nc.scalar.activation(acts[:], gps[:], AF.Tanh)
                nc.vector.scalar_tensor_tensor(
                    out=t1[:], in0=acts[:, sl_i], scalar=1.0,
                    in1=acts[:, sl_g], op0=ALU.add, op1=ALU.mult)
                nc.vector.scalar_tensor_tensor(
                    out=t2[:], in0=acts[:, sl_f], scalar=1.0,
                    in1=c_t[:], op0=ALU.add, op1=ALU.mult)
                nc.vector.scalar_tensor_tensor(
                    out=c_t[:], in0=t2[:], scalar=0.5,
                    in1=t1[:], op0=ALU.mult, op1=ALU.add)
                th = wp.tile([100, GW], F32, tag="th", name="th")
                nc.scalar.activation(th[:], c_t[:], AF.Tanh, scale=0.5)
                nc.vector.scalar_tensor_tensor(
                    out=h_t[:], in0=acts[:, sl_o], scalar=1.0,
                    in1=th[:], op0=ALU.add, op1=ALU.mult)

            ops = pcb.tile([B, 2], F32, tag="sps", name="ops")
            nc.tensor.matmul(ops[:], h_t[:], fco[:], start=True, stop=True)
            out_sb = wp.tile([B, 2], F32, tag="out", name="out")
            nc.scalar.activation(out_sb[:], ops[:], AF.Identity,
                                 bias=cst_sb[0:B, C_FOB:C_FOB + 1])
            nc.sync.dma_start(out.ap(), out_sb[:])

    nc.compile()
    return nc


_CACHE = {}


def _get_nc(B):
    if B not in _CACHE:
        _CACHE[B] = _build(B)
    return _CACHE[B]


def _prep_inputs(B, **inp):
    """Host-side sharding/layout prep (slicing, transposes, weight folds)."""
    f32 = np.float32
    replay = np.asarray(inp["replay_tensor"], f32)[:B]
    BT = B * T
    F = BT // N_CORES
    frames = replay.reshape(BT, 8, 128, 128)
    scal_all = frames[:, 0]                                    # [BT,128,128]
    v0_all = frames[:, CAT_FEATS, 0, 0]                        # [BT,5]

    emb = np.asarray(inp["emb_table"], f32)
    d = emb[1] - emb[0]                                        # [200]
    w1 = np.asarray(inp["cnn1_w"], f32)[:, :, 0, :]            # [964,1928,2]
    lhsT_h = np.zeros((15, 964), f32)
    for k in range(5):
        for dx in range(2):
            lhsT_h[2 * k + dx] = w1[:, CKZ[k], dx]
    for dx in range(2):
        lhsT_h[10 + dx] = w1[:, 1:, dx].sum(axis=1)            # W1sum
    lhsT_h[12] = np.asarray(inp["cnn1_b"], f32)                # bias row
    for dx in range(2):
        lhsT_h[13 + dx] = w1[:, 0, dx]                         # scalar rows

    e0dup3 = np.zeros((3, F * 199), f32)
    for dx in range(2):
        e0dup3[dx] = np.tile(emb[0, dx:dx + 199], F)
    e0dup3[2] = 1.0

    consts = np.zeros((128, NCC), f32)
    consts[:, C_CCW:C_CCW + 16] = np.asarray(
        inp["conv_con_w"], f32).reshape(16)[None, :]
    consts[:, C_CCB] = np.asarray(inp["conv_con_b"], f32).reshape(1)[0]
    consts[0:10, C_DT:C_DT + 199] = np.stack(
        [d[dx:dx + 199] for k in range(5) for dx in range(2)])
    consts[0:2, C_FOB] = np.asarray(inp["fc_out_b"], f32).reshape(2)

    fcc2 = np.zeros((992, 200), f32)
    fcc2[0:961] = np.asarray(inp["fc_con_w"], f32).T
    fcc2[961] = np.asarray(inp["fc_con_b"], f32)
    fcc2 = fcc2.reshape(8, 124, 200).transpose(1, 0, 2).copy()  # [124,8,200]

    def packw(w, b, nk, mt_k):
        """w [M,K,1,2] conv weight -> [128, nchunks, 2, M] with bias row."""
        wT = np.asarray(w, f32)[:, :, 0, :].transpose(2, 1, 0)  # [2,K,M]
        M = wT.shape[2]
        o = np.zeros((128, nk, 2, M), f32)
        for kt, (ks, ksz) in enumerate(mt_k):
            for dx in range(2):
                o[0:ksz, kt, dx] = wT[dx, ks:ks + ksz]
        o[mt_k[-1][1], nk - 1, 0] = np.asarray(b, f32)          # bias row
        return o

    w2p = packw(inp["cnn2_w"], inp["cnn2_b"], 8, _tiles(964))
    w3p = packw(inp["cnn3_w"], inp["cnn3_b"], 4, _tiles(482))
    w4p = packw(inp["cnn4_w"], inp["cnn4_b"], 2, _tiles(241))

    fcnn2 = np.zeros((100, 24, 100), f32)
    fcnn2[:, 0:23, :] = np.asarray(inp["fc_cnn_w"], f32).reshape(
        100, 100, 23).transpose(1, 2, 0)                        # [c,w,o]
    fcnn2[0, 23, :] = np.asarray(inp["fc_cnn_b"], f32)

    # LSTM folds: gate order (i,f,o,g); i,f,o scaled 0.5 (tanh trick);
    # whh additionally 0.5 (H=2h); fco 0.5.
    perm = np.r_[0:100, 100:200, 300:400, 200:300]
    gsc = np.concatenate([np.full(300, 0.5, f32), np.ones(100, f32)])
    wih = np.asarray(inp["lstm_w_ih"], f32)[perm] * gsc[:, None]
    whh = np.asarray(inp["lstm_w_hh"], f32)[perm] * gsc[:, None] * 0.5
    bsum = ((np.asarray(inp["lstm_b_ih"], f32)
             + np.asarray(inp["lstm_b_hh"], f32))[perm] * gsc)
    lstmpk = np.zeros((100, 806), f32)
    lstmpk[:, 0:400] = wih.T
    lstmpk[:, 400:800] = whh.T
    lstmpk[:, 800:804] = bsum.reshape(4, 100).T
    lstmpk[:, 804:806] = np.asarray(inp["fc_out_w"], f32).T * 0.5

    common = dict(consts=consts, lhsT_h=lhsT_h, e0dup3=e0dup3, fcc2=fcc2,
                  w2p=w2p, w3p=w3p, w4p=w4p, fcnn2=fcnn2, lstmpk=lstmpk)

    in_maps = []
    for j in range(N_CORES):
        m = dict(common)
        sc = scal_all[F * j:F * j + F]                          # [F,128,128]
        m["scal4"] = np.ascontiguousarray(
            sc.reshape(F, 64, 2, 128).transpose(1, 2, 0, 3))
        cj = consts.copy()
        cj[0:10, C_V0:C_V0 + F] = np.repeat(
            v0_all[F * j:F * j + F].T, 2, axis=0)               # [10,F]
        m["consts"] = cj
        in_maps.append(m)
    return in_maps


def kernel(**inputs):
    B = int(np.asarray(inputs.get("batch_size", 2)))
    if B <= 0:
        return np.zeros((0, 2), np.float32)
    nc = _get_nc(B)
    in_maps = _prep_inputs(B, **inputs)
    res = run_bass_kernel_spmd(nc, in_maps, list(range(N_CORES)))
    return np.asarray(res.results[0]["out"], np.float32)
